# revision 1
# baseline (speedup 1.0000x reference)
"""GCGRU cell (graph-conv GRU, diffusion order 3) on 8 TRN2 NeuronCores.

Data-parallel over the batch dim (512 per core). Per core, all tensors are
kept channel-on-partition [C, (b, n)]; the node-dim diffusion (z @ M^k,
M = adj.T) transposes 3-batch column groups through the PE transpose
datapath and multiplies against a host-precomputed block-diagonal
[M^1 | M^2 | M^3] so the result lands back in channel-on-partition layout.
The three 1x1 convs are K=768 bf16 PSUM accumulations in six 128-channel
chunks; sigmoid/tanh run on the scalar engine straight out of PSUM with
fused bias. PSUM->SBUF copies are batched across group pairs to amortize
per-op overhead.
"""
import numpy as np
import ml_dtypes

import concourse.bacc as bacc
import concourse.mybir as mybir
from concourse.tile import TileContext
from concourse.bass_utils import run_bass_kernel_spmd

ORDER = 3
B, D_IN, UNITS, NN = 4096, 64, 128, 36
N_CORES = 8
BS = B // N_CORES            # 512 batches per core
F32, BF16 = mybir.dt.float32, mybir.dt.bfloat16

TBM = 24                     # batches per macro tile (8 groups of 3)
CONV_T = 432                 # conv free-dim tile (12 batches)


# cc-index permutation putting the 768 conv input channels into six
# 128-partition chunks: [g1x|x], [h], [gh1], [g2x|g3x], [gh2], [gh3]
def _perm():
    r = lambda a, b: list(range(a, b))
    return (r(192, 256) + r(0, 64)      # C0: g1 x-rows + x (k0)
            + r(64, 192)                # H:  h (k0)
            + r(256, 384)               # GH1: g1 h-rows
            + r(384, 448) + r(576, 640)  # C3: g2 x-rows + g3 x-rows
            + r(448, 576)               # GH2
            + r(640, 768))              # GH3


def _pack_w(W):
    WT = W.T[_perm(), :]                          # [768, 128]
    return np.ascontiguousarray(
        WT.reshape(6, 128, 128).transpose(1, 0, 2).reshape(128, 768)
    ).astype(np.float32)


def _mbd(adj, pack):
    # block-diag [pack*36, 3*pack*36]: cols (k, b_hat, w) of M^k = (adj.T)^k
    M1 = np.ascontiguousarray(adj.T).astype(np.float64)
    Ms = [M1, M1 @ M1, M1 @ M1 @ M1]
    P = pack * 36
    out = np.zeros((P, 3 * P), np.float64)
    for k in range(3):
        for bh in range(pack):
            out[bh * 36:(bh + 1) * 36, k * P + bh * 36:k * P + (bh + 1) * 36] = Ms[k]
    return out.astype(ml_dtypes.bfloat16)


def _build(bs):
    nc = bacc.Bacc("TRN2", target_bir_lowering=False, debug=False,
                   num_devices=N_CORES)
    d_x = nc.declare_dram_parameter("x", [bs, D_IN, NN], F32, isOutput=False)
    d_h = nc.declare_dram_parameter("h", [bs, UNITS, NN], F32, isOutput=False)
    d_wf = nc.declare_dram_parameter("wf", [128, 768], F32, isOutput=False)
    d_wu = nc.declare_dram_parameter("wu", [128, 768], F32, isOutput=False)
    d_wc = nc.declare_dram_parameter("wc", [128, 768], F32, isOutput=False)
    d_bf = nc.declare_dram_parameter("bf", [128, 1], F32, isOutput=False)
    d_bu = nc.declare_dram_parameter("bu", [128, 1], F32, isOutput=False)
    d_bc = nc.declare_dram_parameter("bc", [128, 1], F32, isOutput=False)
    d_m3 = nc.declare_dram_parameter("mbd3", [108, 324], BF16, isOutput=False)
    d_m2 = nc.declare_dram_parameter("mbd2", [72, 216], BF16, isOutput=False)
    d_id = nc.declare_dram_parameter("ident", [128, 128], BF16, isOutput=False)
    d_out = nc.declare_dram_parameter("out", [bs, UNITS, NN], F32, isOutput=True)

    SIG = mybir.ActivationFunctionType.Sigmoid
    TANH = mybir.ActivationFunctionType.Tanh

    with TileContext(nc) as tc:
        with (
            tc.tile_pool(name="consts", bufs=1) as cpool,
            tc.tile_pool(name="macro", bufs=2) as mpool,
            tc.tile_pool(name="small", bufs=3) as spool,
            tc.tile_pool(name="ps_t", bufs=1, space="PSUM") as ps_t,
            tc.tile_pool(name="ps_nm", bufs=1, space="PSUM") as ps_nm,
            tc.tile_pool(name="ps_conv", bufs=2, space="PSUM") as ps_conv,
        ):
            wf = cpool.tile([128, 768], F32, name="wf")
            wu = cpool.tile([128, 768], F32, name="wu")
            wc = cpool.tile([128, 768], F32, name="wc")
            bf = cpool.tile([128, 1], F32, name="bf")
            bu = cpool.tile([128, 1], F32, name="bu")
            bc = cpool.tile([128, 1], F32, name="bc")
            m3 = cpool.tile([108, 324], BF16, name="m3")
            m2 = cpool.tile([72, 216], BF16, name="m2")
            ident = cpool.tile([128, 128], BF16, name="ident")
            for dst, src in ((wf, d_wf), (wu, d_wu), (wc, d_wc), (bf, d_bf),
                             (bu, d_bu), (bc, d_bc), (m3, d_m3), (m2, d_m2),
                             (ident, d_id)):
                nc.sync.dma_start(out=dst[:], in_=src[:])

            wf16 = cpool.tile([128, 768], BF16, name="wf16")
            wu16 = cpool.tile([128, 768], BF16, name="wu16")
            wc16 = cpool.tile([128, 768], BF16, name="wc16")
            nc.vector.tensor_copy(wf16[:], wf[:])
            nc.vector.tensor_copy(wu16[:], wu[:])
            nc.vector.tensor_copy(wc16[:], wc[:])
            id_hi = ident[64:128, 64:128]

            macros = [(m * TBM, TBM) for m in range(bs // TBM)]
            if bs % TBM:
                macros.append((bs - bs % TBM, bs % TBM))

            for b0, nb in macros:
                F = nb * 36
                packs = [3] * (nb // 3)
                if nb % 3 == 2:
                    packs.append(2)
                elif nb % 3 == 1:
                    packs[-1] = 2
                    packs.append(2)
                gstarts = []
                col = 0
                for p in packs:
                    gstarts.append((col, p))
                    col += p * 36
                # pair up equal-sized adjacent groups for batched copies
                pairs = []
                i = 0
                while i < len(gstarts):
                    if (i + 1 < len(gstarts)
                            and gstarts[i][1] == 3 and gstarts[i + 1][1] == 3):
                        pairs.append([gstarts[i], gstarts[i + 1]])
                        i += 2
                    else:
                        pairs.append([gstarts[i]])
                        i += 1

                c0 = mpool.tile([128, F], BF16, tag="c0", name=f"c0_{b0}")
                xx32 = mpool.tile([64, F], F32, tag="xx32", name=f"xx32_{b0}")
                hh = mpool.tile([128, F], F32, tag="hh", name=f"hh_{b0}")
                hh16 = mpool.tile([128, F], BF16, tag="hh16", name=f"hh16_{b0}")
                c3 = mpool.tile([128, F], BF16, tag="c3", name=f"c3_{b0}")
                ghg = mpool.tile([128, 3, F], BF16, tag="ghg", name=f"ghg_{b0}")
                rr = mpool.tile([128, F], F32, tag="rr", name=f"rr_{b0}")
                uu = mpool.tile([128, F], F32, tag="uu", name=f"uu_{b0}")
                rhh = mpool.tile([128, F], F32, tag="rhh", name=f"rhh_{b0}")
                rhh16 = mpool.tile([128, F], BF16, tag="rhh16",
                                   name=f"rhh16_{b0}")
                rhg = mpool.tile([128, 3, F], BF16, tag="rhg", name=f"rhg_{b0}")
                ct = mpool.tile([128, F], F32, tag="ct", name=f"ct_{b0}")
                oo = mpool.tile([128, F], F32, tag="oo", name=f"oo_{b0}")

                bsl = slice(b0, b0 + nb)
                nc.sync.dma_start(
                    out=xx32[:].rearrange("c (b n) -> c b n", b=nb),
                    in_=d_x[bsl].rearrange("b c n -> c b n"))
                nc.sync.dma_start(
                    out=hh[:].rearrange("c (b n) -> c b n", b=nb),
                    in_=d_h[bsl].rearrange("b c n -> c b n"))
                nc.vector.tensor_copy(c0[64:128, :], xx32[:])
                nc.vector.tensor_copy(hh16[:], hh[:])

                # --- diffusion of z = [x; h], one pair of 3-batch groups at
                # a time: PE-transpose both groups into one 2-bank PSUM tile,
                # one batched copy to SBUF, then block-diag node-mix ---
                for pair in pairs:
                    ptz = ps_t.tile([108, 1024], BF16, tag="ptz",
                                    name=f"ptz_{b0}_{pair[0][0]}")
                    zt3 = spool.tile([108, 384], BF16, tag="zt3",
                                     name=f"zt3_{b0}_{pair[0][0]}")
                    pgx = ps_nm.tile([128, 512], F32, tag="pgx",
                                     name=f"pgx_{b0}_{pair[0][0]}")
                    pgh = ps_nm.tile([128, 1024], F32, tag="pgh",
                                     name=f"pgh_{b0}_{pair[0][0]}")
                    for j, (g0, p) in enumerate(pair):
                        P = p * 36
                        zo = j * 512
                        nc.tensor.transpose(ptz[0:P, zo:zo + 64],
                                            c0[64:128, g0:g0 + P], id_hi)
                        nc.tensor.transpose(ptz[0:P, zo + 64:zo + 192],
                                            hh16[:, g0:g0 + P], ident[:])
                    if len(pair) == 2:
                        nc.vector.tensor_copy(
                            zt3[:].rearrange("p (g x) -> p g x", g=2),
                            ptz[:].rearrange("p (g x) -> p g x", g=2)
                            [:, :, 0:192])
                    else:
                        P = pair[0][1] * 36
                        nc.vector.tensor_copy(zt3[0:P, 0:192],
                                              ptz[0:P, 0:192])
                    for j, (g0, p) in enumerate(pair):
                        P = p * 36
                        mbd = m3 if p == 3 else m2
                        so, xo, ho = j * 192, j * 216, j * 512
                        zx = zt3[0:P, so:so + 64]
                        nc.tensor.matmul(pgx[0:64, xo:xo + 2 * P], zx,
                                         mbd[:, 0:2 * P])
                        nc.tensor.matmul(pgx[64:128, xo + P:xo + 2 * P], zx,
                                         mbd[:, 2 * P:3 * P])
                        nc.tensor.matmul(pgh[0:64, ho:ho + 3 * P],
                                         zt3[0:P, so + 64:so + 128], mbd[:])
                        nc.tensor.matmul(pgh[64:128, ho:ho + 3 * P],
                                         zt3[0:P, so + 128:so + 192], mbd[:])
                    gp = pair[0][0]
                    if len(pair) == 2:
                        nc.vector.tensor_copy(
                            c0[0:64, gp:gp + 216].rearrange(
                                "c (g w) -> c g w", g=2),
                            pgx[0:64, 0:432].rearrange(
                                "c (g w) -> c g w", g=2)[:, :, 0:108])
                        nc.vector.tensor_copy(
                            c3[:, gp:gp + 216].rearrange(
                                "c (g w) -> c g w", g=2),
                            pgx[:, 0:432].rearrange(
                                "c (g w) -> c g w", g=2)[:, :, 108:216])
                        nc.scalar.copy(
                            ghg[:, :, gp:gp + 216].rearrange(
                                "c k (g w) -> c k g w", g=2),
                            pgh[:].rearrange(
                                "c (g x) -> c g x", g=2)[:, :, 0:324]
                            .rearrange("c g (k w) -> c k g w", k=3))
                    else:
                        P = pair[0][1] * 36
                        nc.vector.tensor_copy(c0[0:64, gp:gp + P],
                                              pgx[0:64, 0:P])
                        nc.vector.tensor_copy(c3[:, gp:gp + P],
                                              pgx[:, P:2 * P])
                        nc.scalar.copy(
                            ghg[:, :, gp:gp + P],
                            pgh[:, 0:3 * P].rearrange("c (k w) -> c k w", k=3))

                # --- f gate first (unblocks r*h), then u ---
                tiles = [slice(t, min(t + CONV_T, F))
                         for t in range(0, F, CONV_T)]
                fu_rhs = [c0, hh16, ghg, c3, ghg, ghg]
                fu_k = [None, None, 0, None, 1, 2]

                def conv(w_sb, rhs_list, k_list, t, psname):
                    pc = ps_conv.tile([128, 512], F32, tag="pconv",
                                      name=psname)
                    T = t.stop - t.start
                    for i in range(6):
                        rhs = rhs_list[i]
                        ap = (rhs[:, k_list[i], t] if k_list[i] is not None
                              else rhs[:, t])
                        nc.tensor.matmul(pc[:, 0:T],
                                         w_sb[:, 128 * i:128 * (i + 1)],
                                         ap, start=(i == 0), stop=(i == 5))
                    return pc

                for t in tiles:
                    pc = conv(wf16, fu_rhs, fu_k, t, f"pcf_{b0}_{t.start}")
                    nc.scalar.activation(rr[:, t], pc[:, 0:t.stop - t.start],
                                         SIG, bias=bf[:, 0:1])
                nc.vector.tensor_mul(rhh[:], rr[:], hh[:])
                nc.vector.tensor_copy(rhh16[:], rhh[:])
                for t in tiles:
                    pc = conv(wu16, fu_rhs, fu_k, t, f"pcu_{b0}_{t.start}")
                    nc.scalar.activation(uu[:, t], pc[:, 0:t.stop - t.start],
                                         SIG, bias=bu[:, 0:1])

                # --- diffusion of r*h per pair ---
                for pair in pairs:
                    ptr = ps_t.tile([108, 1024], BF16, tag="ptz",
                                    name=f"ptr_{b0}_{pair[0][0]}")
                    rht3 = spool.tile([108, 256], BF16, tag="rht3",
                                      name=f"rht3_{b0}_{pair[0][0]}")
                    for j, (g0, p) in enumerate(pair):
                        P = p * 36
                        nc.tensor.transpose(ptr[0:P, j * 512:j * 512 + 128],
                                            rhh16[:, g0:g0 + P], ident[:])
                    if len(pair) == 2:
                        nc.vector.tensor_copy(
                            rht3[:].rearrange("p (g x) -> p g x", g=2),
                            ptr[:].rearrange("p (g x) -> p g x", g=2)
                            [:, :, 0:128])
                    else:
                        P = pair[0][1] * 36
                        nc.vector.tensor_copy(rht3[0:P, 0:128],
                                              ptr[0:P, 0:128])
                    prh = ps_nm.tile([128, 1024], F32, tag="prh",
                                     name=f"prh_{b0}_{pair[0][0]}")
                    for j, (g0, p) in enumerate(pair):
                        P = p * 36
                        mbd = m3 if p == 3 else m2
                        so, ho = j * 128, j * 512
                        nc.tensor.matmul(prh[0:64, ho:ho + 3 * P],
                                         rht3[0:P, so:so + 64], mbd[:])
                        nc.tensor.matmul(prh[64:128, ho:ho + 3 * P],
                                         rht3[0:P, so + 64:so + 128], mbd[:])
                    gp = pair[0][0]
                    if len(pair) == 2:
                        nc.scalar.copy(
                            rhg[:, :, gp:gp + 216].rearrange(
                                "c k (g w) -> c k g w", g=2),
                            prh[:].rearrange(
                                "c (g x) -> c g x", g=2)[:, :, 0:324]
                            .rearrange("c g (k w) -> c k g w", k=3))
                    else:
                        P = pair[0][1] * 36
                        nc.scalar.copy(
                            rhg[:, :, gp:gp + P],
                            prh[:, 0:3 * P].rearrange("c (k w) -> c k w", k=3))

                # --- candidate gate + output ---
                c_rhs = [c0, rhh16, rhg, c3, rhg, rhg]
                c_k = [None, None, 0, None, 1, 2]
                for t in tiles:
                    pc = conv(wc16, c_rhs, c_k, t, f"pcc_{b0}_{t.start}")
                    nc.scalar.activation(ct[:, t], pc[:, 0:t.stop - t.start],
                                         TANH, bias=bc[:, 0:1])
                # out = c + u * (h - c)
                nc.vector.tensor_sub(oo[:], hh[:], ct[:])
                nc.vector.tensor_mul(oo[:], oo[:], uu[:])
                nc.vector.tensor_add(oo[:], oo[:], ct[:])

                nc.sync.dma_start(
                    out=d_out[bsl].rearrange("b c n -> c b n"),
                    in_=oo[:].rearrange("c (b n) -> c b n", b=nb))
    nc.compile()
    return nc


_CACHE = {}
LAST_RESULTS = None


def _get_nc(bs):
    if bs not in _CACHE:
        _CACHE[bs] = _build(bs)
    return _CACHE[bs]


def kernel(x, h, adj, W_f, b_f, W_u, b_u, W_c, b_c):
    x = np.ascontiguousarray(x, np.float32)
    h = np.ascontiguousarray(h, np.float32)
    consts = {
        "wf": _pack_w(np.asarray(W_f, np.float32)),
        "wu": _pack_w(np.asarray(W_u, np.float32)),
        "wc": _pack_w(np.asarray(W_c, np.float32)),
        "bf": np.asarray(b_f, np.float32).reshape(128, 1),
        "bu": np.asarray(b_u, np.float32).reshape(128, 1),
        "bc": np.asarray(b_c, np.float32).reshape(128, 1),
        "mbd3": _mbd(np.asarray(adj, np.float32), 3),
        "mbd2": _mbd(np.asarray(adj, np.float32), 2),
        "ident": np.eye(128, dtype=ml_dtypes.bfloat16),
    }
    bs = x.shape[0] // N_CORES
    nc = _get_nc(bs)
    in_maps = [
        {"x": x[i * bs:(i + 1) * bs], "h": h[i * bs:(i + 1) * bs], **consts}
        for i in range(N_CORES)
    ]
    res = run_bass_kernel_spmd(nc, in_maps, list(range(N_CORES)))
    global LAST_RESULTS
    LAST_RESULTS = res
    return np.concatenate([res.results[i]["out"] for i in range(N_CORES)], axis=0)

